# revision 44
# baseline (speedup 1.0000x reference)
"""Multi-head attention (B=2, L=2048, D=1024, H=16) on 8 trn2 cores.

Sharding: core c -> (batch b = c//4) x (head-group hg = c%4, 4 heads each).
W_q/W_k/W_v are column-split, W_o row-split; the 4 partial outputs per
batch are summed on the host (plus bo).

Per-core kernel (all matmuls lhsT.T @ rhs, contraction on partitions):
  inputs are host-transposed (xT = x.T, so d_model lands on partitions):
    QT[256,2048] = (Wq_s chunkT).T @ xqT   (accumulate over 8 d-chunks)
    KT, VT likewise.  V = PE-transpose(VT) per head -> V'[Lk, 65]
    (65th column = ones, used to compute the softmax denominator).
  attention per head, scores kept transposed (Lk on partitions):
    ST[128,512] = KT_h_chunk.T @ QT_h_bank          (K = dk = 64)
    P = Exp(ST * (1/sqrt(dk)) + mask_bias)          (one ACT op: scale+mask+exp)
    OT'[65,512] += V'_chunk.T @ P                   (row 64 = denominator)
    OT = OT'[0:64] * broadcast(1/OT'[64])           (DMA partition-broadcast)
  out[2048,1024] = (OT_all chunk).T @ Wo_s  (per-core partial, summed on host)
"""

import sys

for _p in ("/opt/trn_rl_repo",):
    if _p not in sys.path:
        sys.path.insert(0, _p)

import numpy as np

import concourse.bass as bass
import concourse.mybir as mybir
import concourse.tile as tile
from concourse import bacc
from concourse.bass import ts
from concourse.bass_utils import run_bass_kernel_spmd
from concourse.tile_rust import add_dep_helper

F32 = mybir.dt.float32
BF16 = mybir.dt.bfloat16

D_MODEL = 1024
NUM_HEADS = 16
D_K = 64
B = 2
L = 2048
N_CORES = 8
HPC = NUM_HEADS // 4  # heads per core (4)
SCALE = float(np.sqrt(D_K))
MASK_BIAS = -30000.0


def build_nc(L=L, D=D_MODEL, HPC=HPC, mm_dtype=mybir.dt.float32r):
    """Build the per-core Bass program (SPMD: same program, 8 cores).

    Wait-budget discipline (every DMA descriptor and every fp32r matmul
    carries a single hardware sync-wait; compute instructions may carry
    several because walrus splits them):
      * PE writes PSUM, ACT (ScalarE) evacuates PSUM, DVE stays off PSUM,
        so psum slot releases ride the ACT semaphore which the PE already
        tracks through its exp-output waits.
      * DMA-written SBUF slots use bufs=8 so a slot's previous writer sits
        on the same HW ring (WAW implied by ring FIFO); the one remaining
        wait is the readers' engine semaphore.  Explicit order chains keep
        the ring rotation deterministic.
      * softmax normalization broadcasts 1/denom with a K=1 matmul
        (ones.T @ recip_row) instead of a DRAM-bounce DMA.
      * zero "primer" matmuls open each accumulation group so real matmuls
        never join a group start with a data wait.
    """
    DK = D_K
    C = HPC * DK           # attention columns per core (256)
    CT = C // 128          # col tiles (2)
    DC = D // 128          # d_model chunks (8)
    LB = L // 512          # Lq banks (4)
    LT = L // 128          # Lk tiles (16)
    NH = 1                  # one full-width x chunk per d-chunk (bf16: 4KB/row)
    CW = L // NH            # x-chunk width

    MDT = mm_dtype  # dtype for every tensor feeding a matmul

    nc = bacc.Bacc("TRN2", target_bir_lowering=False, debug=False,
                   num_devices=N_CORES)

    xT = {n: nc.dram_tensor(f"x{n}T", [D, L], BF16, kind="ExternalInput").ap()
          for n in ("q", "k", "v")}
    w = {n: nc.dram_tensor(f"w{n}", [D, C], BF16, kind="ExternalInput").ap()
         for n in ("q", "k", "v")}
    wo = nc.dram_tensor("wo", [C, D], BF16, kind="ExternalInput").ap()
    bias = {n: nc.dram_tensor(f"b{n}", [C], F32, kind="ExternalInput").ap()
            for n in ("q", "k", "v")}
    mb = nc.dram_tensor("mb", [128, LT], F32, kind="ExternalInput").ap()
    id2 = nc.dram_tensor("id2", [128, DK], MDT, kind="ExternalInput").ap()
    onesd = nc.dram_tensor("ones", [1, DK], MDT, kind="ExternalInput").ap()
    ztd = nc.dram_tensor("ztc", [128, 128], MDT, kind="ExternalInput").ap()
    vod = nc.dram_tensor("vones", [128, LT * HPC], BF16,
                         kind="ExternalInput").ap()
    out = nc.dram_tensor("partial", [L, D], BF16, kind="ExternalOutput").ap()

    Ident = mybir.ActivationFunctionType.Identity

    with tile.TileContext(nc) as tc:
        with (
            tc.tile_pool(name="consts", bufs=1) as consts,
            tc.tile_pool(name="persist", bufs=1) as persist,
            tc.tile_pool(name="xch", bufs=8) as xch,
            tc.tile_pool(name="work", bufs=6) as work,
            tc.tile_pool(name="pt", bufs=6) as ptp,
            tc.tile_pool(name="ostg", bufs=8) as ostgp,
            tc.tile_pool(name="ps", bufs=8, space="PSUM") as psp,
        ):
            def prime(out_ps, rhs_ap, lhsT=None, start=True, stop=True):
                return nc.tensor.matmul(
                    out_ps, lhsT=(lhsT if lhsT is not None else rhs_ap[:, :1]),
                    rhs=rhs_ap, start=start, stop=stop,
                    skip_group_check=stop is True)

            # PSUM budget (8 banks total, one pool, two tags):
            #   "s2": 2 bufs x [128,1024] f32 (2 banks each) -> 4 banks
            #   "ot": 4 bufs x [128,512]  f32 (1 bank each)  -> 4 banks
            def ps_tile(name=None, dtype=F32):
                return psp.tile([128, 512], dtype, tag="ot", bufs=4,
                                name=name or "pst")

            def ps_wide(name=None, dtype=F32, shape=None):
                return psp.tile(shape or [128, 1024], dtype, tag="s2", bufs=2,
                                name=name or "psw")

            # ---- constants (SWDGE; x/out stay on the HWDGE rings) ----
            w_sb = {}
            for n in ("q", "k", "v"):
                w_sb[n] = consts.tile([128, DC, C], BF16, tag=f"w{n}",
                                      name=f"w{n}_sb")
                nc.gpsimd.dma_start(
                    out=w_sb[n], in_=w[n].rearrange("(c p) n -> p c n", p=128))
            wo_sb = consts.tile([128, CT, D], BF16, tag="wo")
            nc.gpsimd.dma_start(
                out=wo_sb, in_=wo.rearrange("(g p) n -> p g n", p=128))
            b_sb = {}
            for n in ("q", "k", "v"):
                b_sb[n] = consts.tile([128, CT], F32, tag=f"b{n}",
                                      name=f"b{n}_sb")
                nc.gpsimd.dma_start(
                    out=b_sb[n], in_=bias[n].rearrange("(t p) -> p t", p=128))
            mb_sb = consts.tile([128, LT], F32, tag="mb")
            nc.gpsimd.dma_start(out=mb_sb, in_=mb)
            id2_sb = consts.tile([128, DK], MDT, tag="id2")
            nc.gpsimd.dma_start(out=id2_sb, in_=id2)
            v_all = persist.tile([128, LT, HPC, DK + 1], BF16, tag="vall")
            nc.gpsimd.dma_start(
                out=v_all[:, :, :, DK],
                in_=vod.rearrange("p (c h) -> p c h", c=LT))
            ones_sb = consts.tile([1, DK], MDT, tag="ones")
            nc.gpsimd.dma_start(out=ones_sb, in_=onesd)

            # ---- phase 1: projections -> QT/KT/VT [128, CT, L] ----
            # accumulators: ct=0 -> two [128,1024] "s2" tiles (lb pairs),
            # ct=1 -> four [128,512] "ot" tiles; 8 banks total.
            projT = {}
            last_xdma = None
            for n in ("q", "k", "v"):
                # q/k land in bf16 (the score matmul runs 2-3x faster than
                # f32r on hw); v stays f32r for the PE transpose, rounded to
                # bf16 at the v_all evacuation.
                dst = persist.tile([128, CT, L], MDT if n == "v" else BF16,
                                   tag=f"{n}t", name=f"{n}t_sb")
                projT[n] = dst
                wide = [ps_wide(name=f"pw_{n}_{i}") for i in range(2)]
                narrow = [ps_tile(name=f"pn_{n}_{i}") for i in range(LB)]

                def acc_ap(ct, lb):
                    if ct == 0:
                        return wide[lb // 2][:, ts(lb % 2, 512)]
                    return narrow[lb]

                for hc in range(DC * NH):
                    dc, lh = hc // NH, hc % NH
                    xc = xch.tile([128, CW], BF16, tag="xc")
                    xd = nc.sync.dma_start(
                        out=xc, in_=xT[n][ts(dc, 128), ts(lh, CW)])
                    if last_xdma is not None:
                        # deterministic ring rotation (slot i <-> ring i)
                        add_dep_helper(xd.ins, last_xdma.ins, sync=False,
                                       reason="xdma-order")
                    last_xdma = xd
                    for ct in range(CT):
                        for lb2 in range(CW // 512):
                            lb = lh * (CW // 512) + lb2
                            nc.tensor.matmul(
                                acc_ap(ct, lb),
                                lhsT=w_sb[n][:, dc, ts(ct, 128)],
                                rhs=xc[:, ts(lb2, 512)],
                                start=(dc == 0), stop=(dc == DC - 1))
                for lbp in range(2):
                    nc.scalar.activation(
                        dst[:, 0, ts(lbp, 1024)], wide[lbp], Ident,
                        bias=b_sb[n][:, 0:1])
                for lb in range(LB):
                    nc.scalar.activation(
                        dst[:, 1, ts(lb, 512)], narrow[lb], Ident,
                        bias=b_sb[n][:, 1:2])

            # ---- phase 1b: V' = [V | ones] in natural [Lk, 65] layout ----
            for h in range(HPC):
                g, po = h // 2, 64 * (h % 2)
                for c in range(LT):
                    pst = ps_tile(name="pst", dtype=MDT)
                    nc.tensor.transpose(
                        pst[:, :DK],
                        projT["v"][po:po + DK, g, ts(c, 128)],
                        id2_sb[po:po + DK, :])
                    nc.vector.tensor_copy(v_all[:, c, h, 0:DK], pst[:, :DK])

            # ---- phase 2: attention, head-pair packed ----
            # The two heads of a g-group sit on partitions 0-63 / 64-127 of
            # projT, so their K=64 score matmuls carry tile_position (0,0) /
            # (64,0) and execute CONCURRENTLY in the PE array (row tiling).
            # A half-array matmul stream reads as low PE activity and leaves
            # the HAM clock gate throttled at 1.2 GHz; the packed pair fills
            # all 128 rows.  One 1024-wide EXP covers both heads (the mask
            # bias depends only on the k-chunk c, not the head).
            ot_sb = persist.tile([128, CT, L], BF16, tag="ot")
            Exp = mybir.ActivationFunctionType.Exp

            def tail_pre(otp):
                # Normalization front half (all DVE): pull the denominator
                # row and O to SBUF (releasing the psum bank), ~51-ULP
                # reciprocal, round to f32r for the broadcast matmul.
                dn = work.tile([1, 512], F32, tag="dn")
                nc.vector.tensor_copy(dn, otp[DK:DK + 1, :])
                ot_c = work.tile([64, 512], F32, tag="otc")
                nc.vector.tensor_copy(ot_c, otp[0:DK, :])
                rc = work.tile([1, 512], F32, tag="rc")
                nc.vector.reciprocal_approx_fast(rc, dn)
                rc_r = work.tile([1, 512], MDT, tag="rcr")
                with nc.allow_low_precision(
                        reason="~18-bit reciprocal feeds the "
                               "broadcast matmul; |denom|>=1"):
                    nc.vector.tensor_copy(rc_r, rc)
                return ot_c, rc_r

            def tail_post(g, lb, po, otp, ot_c, rc_r):
                # Back half: broadcast 1/denom into the already-copied-out
                # psum bank with a K=1 matmul, multiply on DVE into ot_sb.
                # Deferred past the next pass's first EXP so the in-order PE
                # queue never stalls waiting on the DVE reciprocal chain.
                nc.tensor.matmul(otp[:DK, :], lhsT=ones_sb,
                                 rhs=rc_r, start=True, stop=True)
                nc.vector.tensor_mul(
                    ot_sb[po:po + DK, g, ts(lb, 512)],
                    ot_c, otp[0:DK, :])

            # phase 3 is emitted in (t, half) units; the g=1 passes interleave
            # the units for the previous lb chunk into their PE/DVE slack
            # (the pass schedule is ACT-bound), leaving only the last chunk
            # exposed after the attention loop.
            WOW = min(512, D)
            p3_state = {"last_odma": None, "ostg": {}}

            def p3_unit(t, half):
                wps = psp.tile([128, 512], F32, tag="s2", bufs=2, name="wps")
                for gg in range(CT):
                    nc.tensor.matmul(
                        wps[:, :WOW], lhsT=ot_sb[:, gg, ts(t, 128)],
                        rhs=wo_sb[:, gg, ts(half, WOW)],
                        start=(gg == 0), stop=(gg == CT - 1))
                if t not in p3_state["ostg"]:
                    p3_state["ostg"][t] = ostgp.tile(
                        [128, D], BF16, tag="os", name="ostg")
                nc.vector.tensor_copy(
                    p3_state["ostg"][t][:, ts(half, WOW)], wps[:, :WOW])
                if half == D // WOW - 1:
                    od = nc.sync.dma_start(out=out[ts(t, 128), :],
                                           in_=p3_state["ostg"].pop(t))
                    if p3_state["last_odma"] is not None:
                        add_dep_helper(od.ins, p3_state["last_odma"].ins,
                                       sync=False, reason="odma-order")
                    p3_state["last_odma"] = od

            pending = []
            for g in range(CT):
                hA, hB = 2 * g, 2 * g + 1
                for lb in range(LB):
                    otA = ps_tile(name="otA")
                    otB = ps_tile(name="otB")
                    pend = None
                    for c in range(LT):
                        s2 = ps_wide(name="s2")
                        nc.tensor.matmul(
                            s2[:, 0:512],
                            lhsT=projT["k"][0:DK, g, ts(c, 128)],
                            rhs=projT["q"][0:DK, g, ts(lb, 512)],
                            start=True, stop=True)
                        nc.tensor.matmul(
                            s2[:, 512:1024],
                            lhsT=projT["k"][DK:2 * DK, g, ts(c, 128)],
                            rhs=projT["q"][DK:2 * DK, g, ts(lb, 512)],
                            start=True, stop=True)
                        p2 = ptp.tile([128, 1024], BF16, tag="p")
                        nc.scalar.activation(
                            p2, s2, Exp,
                            bias=mb_sb[:, c:c + 1], scale=1.0 / SCALE)
                        if c == 1:
                            for args in pending:
                                tail_post(*args)
                            pending = []
                        if g == 1 and lb >= 1 and 1 <= c <= 8:
                            tp3 = 4 * (lb - 1) + (c - 1) // 2
                            p3_unit(tp3, (c - 1) % 2)
                        if pend is not None:
                            c0, p0 = pend
                            nc.tensor.matmul(
                                otA[:DK + 1, :],
                                lhsT=v_all[:, c0, hA, :], rhs=p0[:, 0:512],
                                start=(c0 == 0), stop=(c0 == LT - 1))
                            nc.tensor.matmul(
                                otB[:DK + 1, :],
                                lhsT=v_all[:, c0, hB, :], rhs=p0[:, 512:1024],
                                start=(c0 == 0), stop=(c0 == LT - 1))
                        pend = (c, p2)
                    c0, p0 = pend
                    nc.tensor.matmul(
                        otA[:DK + 1, :], lhsT=v_all[:, c0, hA, :],
                        rhs=p0[:, 0:512], start=(c0 == 0), stop=True)
                    nc.tensor.matmul(
                        otB[:DK + 1, :], lhsT=v_all[:, c0, hB, :],
                        rhs=p0[:, 512:1024], start=(c0 == 0), stop=True)
                    for po, otp in ((0, otA), (64, otB)):
                        ot_c, rc_r = tail_pre(otp)
                        pending.append((g, lb, po, otp, ot_c, rc_r))
            for args in pending:
                tail_post(*args)
            for c3 in range(8):
                p3_unit(12 + c3 // 2, c3 % 2)


    nc.compile()   # bacc lowering: event sems split multi-wait instructions
    return nc


def _strip_implied_dma_ring_waits(nc):
    """Drop DMA ring-semaphore waits that are implied by a compute-engine
    wait on the same descriptor.

    A recycled DMA-written SBUF slot gets two waits: the readers' engine
    semaphore (slot release) and the previous writer's DMA-ring semaphore
    (WAW).  The readers themselves data-waited on that previous DMA, so
    release >= WAW always; but DMA descriptors carry a single hardware
    sync-wait, so Tile's conservative pair fails walrus codegen.  Keep the
    engine wait, drop the ring wait.  Applied only to the x-chunk loads and
    output-staging stores, whose only DMA-semaphore deps are these WAW /
    WAR-release edges (their data comes from DRAM inputs or compute-engine
    writes, never from another DMA).
    """
    import concourse.mybir as _mb
    for ins in nc.inst_map.values():
        if type(ins).__name__ != "InstDMACopy":
            continue
        if not ins.outs:
            continue
        memref = getattr(ins.outs[0], "memref", "") or ""
        src_ref = getattr(ins.ins[0], "memref", "") if ins.ins else ""
        if not (memref.startswith("xc_") or (src_ref or "").startswith("ostg")):
            continue
        si = ins.sync_info
        if not si or not si.on_wait or len(si.on_wait) < 2:
            continue
        eng = [w for w in si.on_wait
               if not (w.ant_name or "").startswith(("DMAHW", "DMASW"))]
        if not eng:
            continue
        ins.sync_info = _mb.SyncInfo(on_wait=eng, on_update=list(si.on_update))


def make_in_maps(query, key, value, mask, Wq, bq, Wk, bk, Wv, bv, Wo, bo,
                 L=L, D=D_MODEL, HPC=HPC):
    """Host-side sharding: per-core input dicts."""
    DK = D_K
    C = HPC * DK
    LT = L // 128
    import ml_dtypes
    bf16 = ml_dtypes.bfloat16
    id2 = np.ascontiguousarray(
        np.tile(np.eye(DK, dtype=np.float32), (2, 1)))
    ones = np.ones((1, DK), np.float32)
    ztc = np.zeros((128, 128), np.float32)
    vones = np.ones((128, (L // 128) * HPC), bf16)
    in_maps = []
    xTs = {}
    mbs = {}
    n_cores = (query.shape[0]) * (D // C)
    groups_per_batch = D // C
    for b in range(query.shape[0]):
        xTs[b] = {
            "q": np.ascontiguousarray(query[b].T).astype(bf16),
            "k": np.ascontiguousarray(key[b].T).astype(bf16),
            "v": np.ascontiguousarray(value[b].T).astype(bf16),
        }
        mbf = np.where(mask[b, 0], np.float32(MASK_BIAS), np.float32(0.0))
        mbs[b] = np.ascontiguousarray(
            mbf.reshape(LT, 128).T.astype(np.float32))
    for c in range(n_cores):
        b, hg = divmod(c, groups_per_batch)
        sl = slice(hg * C, (hg + 1) * C)
        in_maps.append({
            "xqT": xTs[b]["q"], "xkT": xTs[b]["k"], "xvT": xTs[b]["v"],
            "wq": np.ascontiguousarray(Wq[:, sl]).astype(bf16),
            "wk": np.ascontiguousarray(Wk[:, sl]).astype(bf16),
            "wv": np.ascontiguousarray(Wv[:, sl]).astype(bf16),
            "wo": np.ascontiguousarray(Wo[sl, :]).astype(bf16),
            "bq": np.ascontiguousarray(bq[sl]),
            "bk": np.ascontiguousarray(bk[sl]),
            "bv": np.ascontiguousarray(bv[sl]),
            "mb": mbs[b],
            "id2": id2,
            "ones": ones, "ztc": ztc, "vones": vones,
        })
    return in_maps


_NC_CACHE = {}


def _get_nc(mm_dtype=mybir.dt.float32r):
    key = str(mm_dtype)
    if key not in _NC_CACHE:
        _NC_CACHE[key] = build_nc(mm_dtype=mm_dtype)
    return _NC_CACHE[key]


def run(inputs, mm_dtype=mybir.dt.float32r, trace=False):
    """Run on 8 cores; returns (full_output, BassKernelResults)."""
    inputs = {k: np.asarray(v) for k, v in inputs.items()}
    nc = _get_nc(mm_dtype)
    in_maps = make_in_maps(**inputs)
    res = run_bass_kernel_spmd(nc, in_maps, list(range(N_CORES)), trace=trace)
    groups_per_batch = N_CORES // B
    out = np.zeros((B, L, D_MODEL), np.float32)
    for b in range(B):
        acc = np.zeros((L, D_MODEL), np.float32)
        for hg in range(groups_per_batch):
            acc += res.results[b * groups_per_batch + hg]["partial"].astype(
                np.float32)
        out[b] = acc + inputs["bo"][None, :]
    return out, res


def kernel(**inputs) -> np.ndarray:
    out, _ = run(inputs)
    return out



# revision 47
# speedup vs baseline: 1.0504x; 1.0504x over previous
"""Multi-head attention (B=2, L=2048, D=1024, H=16) on 8 trn2 cores.

Sharding: core c -> (batch b = c//4) x (head-group hg = c%4, 4 heads each).
W_q/W_k/W_v are column-split, W_o row-split; the 4 partial outputs per
batch are summed on the host (plus bo).

Per-core kernel (all matmuls lhsT.T @ rhs, contraction on partitions):
  inputs are host-transposed (xT = x.T, so d_model lands on partitions):
    QT[256,2048] = (Wq_s chunkT).T @ xqT   (accumulate over 8 d-chunks)
    KT, VT likewise.  V = PE-transpose(VT) per head -> V'[Lk, 65]
    (65th column = ones, used to compute the softmax denominator).
  attention per head, scores kept transposed (Lk on partitions):
    ST[128,512] = KT_h_chunk.T @ QT_h_bank          (K = dk = 64)
    P = Exp(ST * (1/sqrt(dk)) + mask_bias)          (one ACT op: scale+mask+exp)
    OT'[65,512] += V'_chunk.T @ P                   (row 64 = denominator)
    OT = OT'[0:64] * broadcast(1/OT'[64])           (DMA partition-broadcast)
  out[2048,1024] = (OT_all chunk).T @ Wo_s  (per-core partial, summed on host)
"""

import sys

for _p in ("/opt/trn_rl_repo",):
    if _p not in sys.path:
        sys.path.insert(0, _p)

import numpy as np

import concourse.bass as bass
import concourse.mybir as mybir
import concourse.tile as tile
from concourse import bacc
from concourse.bass import ts
from concourse.bass_utils import run_bass_kernel_spmd
from concourse.tile_rust import add_dep_helper

F32 = mybir.dt.float32
BF16 = mybir.dt.bfloat16

D_MODEL = 1024
NUM_HEADS = 16
D_K = 64
B = 2
L = 2048
N_CORES = 8
HPC = NUM_HEADS // 4  # heads per core (4)
SCALE = float(np.sqrt(D_K))
MASK_BIAS = -30000.0


def build_nc(L=L, D=D_MODEL, HPC=HPC, mm_dtype=mybir.dt.float32r):
    """Build the per-core Bass program (SPMD: same program, 8 cores).

    Wait-budget discipline (every DMA descriptor and every fp32r matmul
    carries a single hardware sync-wait; compute instructions may carry
    several because walrus splits them):
      * PE writes PSUM, ACT (ScalarE) evacuates PSUM, DVE stays off PSUM,
        so psum slot releases ride the ACT semaphore which the PE already
        tracks through its exp-output waits.
      * DMA-written SBUF slots use bufs=8 so a slot's previous writer sits
        on the same HW ring (WAW implied by ring FIFO); the one remaining
        wait is the readers' engine semaphore.  Explicit order chains keep
        the ring rotation deterministic.
      * softmax normalization broadcasts 1/denom with a K=1 matmul
        (ones.T @ recip_row) instead of a DRAM-bounce DMA.
      * zero "primer" matmuls open each accumulation group so real matmuls
        never join a group start with a data wait.
    """
    DK = D_K
    C = HPC * DK           # attention columns per core (256)
    CT = C // 128          # col tiles (2)
    DC = D // 128          # d_model chunks (8)
    LB = L // 512          # Lq banks (4)
    LT = L // 128          # Lk tiles (16)
    NH = 1                  # one full-width x chunk per d-chunk (bf16: 4KB/row)
    CW = L // NH            # x-chunk width

    MDT = mm_dtype  # dtype for every tensor feeding a matmul

    nc = bacc.Bacc("TRN2", target_bir_lowering=False, debug=False,
                   num_devices=N_CORES)

    xT = {n: nc.dram_tensor(f"x{n}T", [D, L], BF16, kind="ExternalInput").ap()
          for n in ("q", "k", "v")}
    w = {n: nc.dram_tensor(f"w{n}", [D, C], BF16, kind="ExternalInput").ap()
         for n in ("q", "k", "v")}
    wo = nc.dram_tensor("wo", [C, D], BF16, kind="ExternalInput").ap()
    bias = {n: nc.dram_tensor(f"b{n}", [C], F32, kind="ExternalInput").ap()
            for n in ("q", "k", "v")}
    mb = nc.dram_tensor("mb", [128, LT], F32, kind="ExternalInput").ap()
    id2 = nc.dram_tensor("id2", [128, DK], MDT, kind="ExternalInput").ap()
    onesd = nc.dram_tensor("ones", [1, DK], MDT, kind="ExternalInput").ap()
    ztd = nc.dram_tensor("ztc", [128, 128], MDT, kind="ExternalInput").ap()
    vod = nc.dram_tensor("vones", [128, LT * HPC], BF16,
                         kind="ExternalInput").ap()
    out = nc.dram_tensor("partial", [L, D], BF16, kind="ExternalOutput").ap()

    Ident = mybir.ActivationFunctionType.Identity

    with tile.TileContext(nc) as tc:
        with (
            tc.tile_pool(name="consts", bufs=1) as consts,
            tc.tile_pool(name="persist", bufs=1) as persist,
            tc.tile_pool(name="xch", bufs=8) as xch,
            tc.tile_pool(name="work", bufs=6) as work,
            tc.tile_pool(name="pt", bufs=6) as ptp,
            tc.tile_pool(name="ostg", bufs=8) as ostgp,
            tc.tile_pool(name="ps", bufs=8, space="PSUM") as psp,
        ):
            def prime(out_ps, rhs_ap, lhsT=None, start=True, stop=True):
                return nc.tensor.matmul(
                    out_ps, lhsT=(lhsT if lhsT is not None else rhs_ap[:, :1]),
                    rhs=rhs_ap, start=start, stop=stop,
                    skip_group_check=stop is True)

            # PSUM budget (8 banks total, one pool, two tags):
            #   "s2": 2 bufs x [128,1024] f32 (2 banks each) -> 4 banks
            #   "ot": 4 bufs x [128,512]  f32 (1 bank each)  -> 4 banks
            def ps_tile(name=None, dtype=F32):
                return psp.tile([128, 512], dtype, tag="ot", bufs=4,
                                name=name or "pst")

            def ps_wide(name=None, dtype=F32, shape=None):
                return psp.tile(shape or [128, 1024], dtype, tag="s2", bufs=2,
                                name=name or "psw")

            # ---- constants (SWDGE; x/out stay on the HWDGE rings) ----
            w_sb = {}
            for n in ("q", "k", "v"):
                w_sb[n] = consts.tile([128, DC, C], BF16, tag=f"w{n}",
                                      name=f"w{n}_sb")
                nc.gpsimd.dma_start(
                    out=w_sb[n], in_=w[n].rearrange("(c p) n -> p c n", p=128))
            wo_sb = consts.tile([128, CT, D], BF16, tag="wo")
            nc.gpsimd.dma_start(
                out=wo_sb, in_=wo.rearrange("(g p) n -> p g n", p=128))
            b_sb = {}
            for n in ("q", "k", "v"):
                b_sb[n] = consts.tile([128, CT], F32, tag=f"b{n}",
                                      name=f"b{n}_sb")
                nc.gpsimd.dma_start(
                    out=b_sb[n], in_=bias[n].rearrange("(t p) -> p t", p=128))
            mb_sb = consts.tile([128, LT], F32, tag="mb")
            nc.gpsimd.dma_start(out=mb_sb, in_=mb)
            id2_sb = consts.tile([128, DK], MDT, tag="id2")
            nc.gpsimd.dma_start(out=id2_sb, in_=id2)
            v_all = persist.tile([128, LT, HPC, DK + 1], BF16, tag="vall")
            nc.gpsimd.dma_start(
                out=v_all[:, :, :, DK],
                in_=vod.rearrange("p (c h) -> p c h", c=LT))
            ones_sb = consts.tile([1, DK], MDT, tag="ones")
            nc.gpsimd.dma_start(out=ones_sb, in_=onesd)

            # ---- phase 1: projections -> QT/KT/VT [128, CT, L] ----
            # accumulators: ct=0 -> two [128,1024] "s2" tiles (lb pairs),
            # ct=1 -> four [128,512] "ot" tiles; 8 banks total.
            projT = {}
            last_xdma = None
            for n in ("q", "k", "v"):
                # q/k land in bf16 (the score matmul runs 2-3x faster than
                # f32r on hw); v stays f32r for the PE transpose, rounded to
                # bf16 at the v_all evacuation.
                dst = persist.tile([128, CT, L], MDT if n == "v" else BF16,
                                   tag=f"{n}t", name=f"{n}t_sb")
                projT[n] = dst
                wide = [ps_wide(name=f"pw_{n}_{i}") for i in range(2)]
                narrow = [ps_tile(name=f"pn_{n}_{i}") for i in range(LB)]

                def acc_ap(ct, lb):
                    if ct == 0:
                        return wide[lb // 2][:, ts(lb % 2, 512)]
                    return narrow[lb]

                for hc in range(DC * NH):
                    dc, lh = hc // NH, hc % NH
                    xc = xch.tile([128, CW], BF16, tag="xc")
                    xd = nc.sync.dma_start(
                        out=xc, in_=xT[n][ts(dc, 128), ts(lh, CW)])
                    if last_xdma is not None:
                        # deterministic ring rotation (slot i <-> ring i)
                        add_dep_helper(xd.ins, last_xdma.ins, sync=False,
                                       reason="xdma-order")
                    last_xdma = xd
                    for ct in range(CT):
                        for lb2 in range(CW // 512):
                            lb = lh * (CW // 512) + lb2
                            nc.tensor.matmul(
                                acc_ap(ct, lb),
                                lhsT=w_sb[n][:, dc, ts(ct, 128)],
                                rhs=xc[:, ts(lb2, 512)],
                                start=(dc == 0), stop=(dc == DC - 1))
                for lbp in range(2):
                    nc.scalar.activation(
                        dst[:, 0, ts(lbp, 1024)], wide[lbp], Ident,
                        bias=b_sb[n][:, 0:1])
                for lb in range(LB):
                    nc.scalar.activation(
                        dst[:, 1, ts(lb, 512)], narrow[lb], Ident,
                        bias=b_sb[n][:, 1:2])

            # ---- phase 1b: V' = [V | ones] in natural [Lk, 65] layout ----
            for h in range(HPC):
                g, po = h // 2, 64 * (h % 2)
                for c in range(LT):
                    pst = ps_tile(name="pst", dtype=MDT)
                    nc.tensor.transpose(
                        pst[:, :DK],
                        projT["v"][po:po + DK, g, ts(c, 128)],
                        id2_sb[po:po + DK, :])
                    nc.vector.tensor_copy(v_all[:, c, h, 0:DK], pst[:, :DK])

            # ---- phase 2: attention, head-pair packed ----
            # The two heads of a g-group sit on partitions 0-63 / 64-127 of
            # projT, so their K=64 score matmuls carry tile_position (0,0) /
            # (64,0) and execute CONCURRENTLY in the PE array (row tiling).
            # A half-array matmul stream reads as low PE activity and leaves
            # the HAM clock gate throttled at 1.2 GHz; the packed pair fills
            # all 128 rows.  One 1024-wide EXP covers both heads (the mask
            # bias depends only on the k-chunk c, not the head).
            ot_sb = persist.tile([128, CT, L], BF16, tag="ot")
            Exp = mybir.ActivationFunctionType.Exp

            def tail_pre(otp):
                # Normalization front half (all DVE): pull the denominator
                # row and O to SBUF (releasing the psum bank), ~51-ULP
                # reciprocal, round to f32r for the broadcast matmul.
                dn = work.tile([1, 512], F32, tag="dn")
                nc.vector.tensor_copy(dn, otp[DK:DK + 1, :])
                ot_c = work.tile([64, 512], F32, tag="otc")
                nc.vector.tensor_copy(ot_c, otp[0:DK, :])
                rc = work.tile([1, 512], F32, tag="rc")
                nc.vector.reciprocal_approx_fast(rc, dn)
                rc_r = work.tile([1, 512], MDT, tag="rcr")
                with nc.allow_low_precision(
                        reason="~18-bit reciprocal feeds the "
                               "broadcast matmul; |denom|>=1"):
                    nc.vector.tensor_copy(rc_r, rc)
                return ot_c, rc_r

            def tail_post(g, lb, po, otp, ot_c, rc_r):
                # Back half: broadcast 1/denom into the already-copied-out
                # psum bank with a K=1 matmul, multiply on DVE into ot_sb.
                # Deferred past the next pass's first EXP so the in-order PE
                # queue never stalls waiting on the DVE reciprocal chain.
                nc.tensor.matmul(otp[:DK, :], lhsT=ones_sb,
                                 rhs=rc_r, start=True, stop=True)
                nc.vector.tensor_mul(
                    ot_sb[po:po + DK, g, ts(lb, 512)],
                    ot_c, otp[0:DK, :])

            # phase 3 is emitted in (t, half) units; the g=1 passes interleave
            # the units for the previous lb chunk into their PE/DVE slack
            # (the pass schedule is ACT-bound), leaving only the last chunk
            # exposed after the attention loop.
            WOW = min(512, D)
            p3_state = {"last_odma": None, "ostg": {}}

            def p3_unit(t, half):
                wps = psp.tile([128, 512], F32, tag="s2", bufs=2, name="wps")
                for gg in range(CT):
                    nc.tensor.matmul(
                        wps[:, :WOW], lhsT=ot_sb[:, gg, ts(t, 128)],
                        rhs=wo_sb[:, gg, ts(half, WOW)],
                        start=(gg == 0), stop=(gg == CT - 1))
                if t not in p3_state["ostg"]:
                    p3_state["ostg"][t] = ostgp.tile(
                        [128, D], BF16, tag="os", name="ostg")
                # alternate the psum evacuation between ACT and DVE so the
                # two engines drain the 2-slot wps ring in parallel
                dst = p3_state["ostg"][t][:, ts(half, WOW)]
                if half % 2 == 0:
                    nc.vector.tensor_copy(dst, wps[:, :WOW])
                else:
                    nc.scalar.copy(out=dst, in_=wps[:, :WOW])
                if half == D // WOW - 1:
                    od = nc.sync.dma_start(out=out[ts(t, 128), :],
                                           in_=p3_state["ostg"].pop(t))
                    if p3_state["last_odma"] is not None:
                        add_dep_helper(od.ins, p3_state["last_odma"].ins,
                                       sync=False, reason="odma-order")
                    p3_state["last_odma"] = od

            pending = []
            for g in range(CT):
                hA, hB = 2 * g, 2 * g + 1
                for lb in range(LB):
                    otA = ps_tile(name="otA")
                    otB = ps_tile(name="otB")
                    pend = None
                    for c in range(LT):
                        s2 = ps_wide(name="s2")
                        nc.tensor.matmul(
                            s2[:, 0:512],
                            lhsT=projT["k"][0:DK, g, ts(c, 128)],
                            rhs=projT["q"][0:DK, g, ts(lb, 512)],
                            start=True, stop=True)
                        nc.tensor.matmul(
                            s2[:, 512:1024],
                            lhsT=projT["k"][DK:2 * DK, g, ts(c, 128)],
                            rhs=projT["q"][DK:2 * DK, g, ts(lb, 512)],
                            start=True, stop=True)
                        p2 = ptp.tile([128, 1024], BF16, tag="p")
                        nc.scalar.activation(
                            p2, s2, Exp,
                            bias=mb_sb[:, c:c + 1], scale=1.0 / SCALE)
                        if c == 1:
                            for args in pending:
                                tail_post(*args)
                            pending = []
                        if pend is not None:
                            c0, p0 = pend
                            nc.tensor.matmul(
                                otA[:DK + 1, :],
                                lhsT=v_all[:, c0, hA, :], rhs=p0[:, 0:512],
                                start=(c0 == 0), stop=(c0 == LT - 1))
                            nc.tensor.matmul(
                                otB[:DK + 1, :],
                                lhsT=v_all[:, c0, hB, :], rhs=p0[:, 512:1024],
                                start=(c0 == 0), stop=(c0 == LT - 1))
                        pend = (c, p2)
                    c0, p0 = pend
                    nc.tensor.matmul(
                        otA[:DK + 1, :], lhsT=v_all[:, c0, hA, :],
                        rhs=p0[:, 0:512], start=(c0 == 0), stop=True)
                    nc.tensor.matmul(
                        otB[:DK + 1, :], lhsT=v_all[:, c0, hB, :],
                        rhs=p0[:, 512:1024], start=(c0 == 0), stop=True)
                    for po, otp in ((0, otA), (64, otB)):
                        ot_c, rc_r = tail_pre(otp)
                        pending.append((g, lb, po, otp, ot_c, rc_r))
            for args in pending:
                tail_post(*args)
            for t3 in range(LT):
                for h3 in range(D // WOW):
                    p3_unit(t3, h3)


    nc.compile()   # bacc lowering: event sems split multi-wait instructions
    return nc


def _strip_implied_dma_ring_waits(nc):
    """Drop DMA ring-semaphore waits that are implied by a compute-engine
    wait on the same descriptor.

    A recycled DMA-written SBUF slot gets two waits: the readers' engine
    semaphore (slot release) and the previous writer's DMA-ring semaphore
    (WAW).  The readers themselves data-waited on that previous DMA, so
    release >= WAW always; but DMA descriptors carry a single hardware
    sync-wait, so Tile's conservative pair fails walrus codegen.  Keep the
    engine wait, drop the ring wait.  Applied only to the x-chunk loads and
    output-staging stores, whose only DMA-semaphore deps are these WAW /
    WAR-release edges (their data comes from DRAM inputs or compute-engine
    writes, never from another DMA).
    """
    import concourse.mybir as _mb
    for ins in nc.inst_map.values():
        if type(ins).__name__ != "InstDMACopy":
            continue
        if not ins.outs:
            continue
        memref = getattr(ins.outs[0], "memref", "") or ""
        src_ref = getattr(ins.ins[0], "memref", "") if ins.ins else ""
        if not (memref.startswith("xc_") or (src_ref or "").startswith("ostg")):
            continue
        si = ins.sync_info
        if not si or not si.on_wait or len(si.on_wait) < 2:
            continue
        eng = [w for w in si.on_wait
               if not (w.ant_name or "").startswith(("DMAHW", "DMASW"))]
        if not eng:
            continue
        ins.sync_info = _mb.SyncInfo(on_wait=eng, on_update=list(si.on_update))


def make_in_maps(query, key, value, mask, Wq, bq, Wk, bk, Wv, bv, Wo, bo,
                 L=L, D=D_MODEL, HPC=HPC):
    """Host-side sharding: per-core input dicts."""
    DK = D_K
    C = HPC * DK
    LT = L // 128
    import ml_dtypes
    bf16 = ml_dtypes.bfloat16
    id2 = np.ascontiguousarray(
        np.tile(np.eye(DK, dtype=np.float32), (2, 1)))
    ones = np.ones((1, DK), np.float32)
    ztc = np.zeros((128, 128), np.float32)
    vones = np.ones((128, (L // 128) * HPC), bf16)
    in_maps = []
    xTs = {}
    mbs = {}
    n_cores = (query.shape[0]) * (D // C)
    groups_per_batch = D // C
    for b in range(query.shape[0]):
        xTs[b] = {
            "q": np.ascontiguousarray(query[b].T).astype(bf16),
            "k": np.ascontiguousarray(key[b].T).astype(bf16),
            "v": np.ascontiguousarray(value[b].T).astype(bf16),
        }
        mbf = np.where(mask[b, 0], np.float32(MASK_BIAS), np.float32(0.0))
        mbs[b] = np.ascontiguousarray(
            mbf.reshape(LT, 128).T.astype(np.float32))
    for c in range(n_cores):
        b, hg = divmod(c, groups_per_batch)
        sl = slice(hg * C, (hg + 1) * C)
        in_maps.append({
            "xqT": xTs[b]["q"], "xkT": xTs[b]["k"], "xvT": xTs[b]["v"],
            "wq": np.ascontiguousarray(Wq[:, sl]).astype(bf16),
            "wk": np.ascontiguousarray(Wk[:, sl]).astype(bf16),
            "wv": np.ascontiguousarray(Wv[:, sl]).astype(bf16),
            "wo": np.ascontiguousarray(Wo[sl, :]).astype(bf16),
            "bq": np.ascontiguousarray(bq[sl]),
            "bk": np.ascontiguousarray(bk[sl]),
            "bv": np.ascontiguousarray(bv[sl]),
            "mb": mbs[b],
            "id2": id2,
            "ones": ones, "ztc": ztc, "vones": vones,
        })
    return in_maps


_NC_CACHE = {}


def _get_nc(mm_dtype=mybir.dt.float32r):
    key = str(mm_dtype)
    if key not in _NC_CACHE:
        _NC_CACHE[key] = build_nc(mm_dtype=mm_dtype)
    return _NC_CACHE[key]


def run(inputs, mm_dtype=mybir.dt.float32r, trace=False):
    """Run on 8 cores; returns (full_output, BassKernelResults)."""
    inputs = {k: np.asarray(v) for k, v in inputs.items()}
    nc = _get_nc(mm_dtype)
    in_maps = make_in_maps(**inputs)
    res = run_bass_kernel_spmd(nc, in_maps, list(range(N_CORES)), trace=trace)
    groups_per_batch = N_CORES // B
    out = np.zeros((B, L, D_MODEL), np.float32)
    for b in range(B):
        acc = np.zeros((L, D_MODEL), np.float32)
        for hg in range(groups_per_batch):
            acc += res.results[b * groups_per_batch + hg]["partial"].astype(
                np.float32)
        out[b] = acc + inputs["bo"][None, :]
    return out, res


def kernel(**inputs) -> np.ndarray:
    out, _ = run(inputs)
    return out



# revision 48
# speedup vs baseline: 1.1011x; 1.0483x over previous
"""Multi-head attention (B=2, L=2048, D=1024, H=16) on 8 trn2 cores.

Sharding: core c -> (batch b = c//4) x (head-group hg = c%4, 4 heads each).
W_q/W_k/W_v are column-split, W_o row-split; the 4 partial outputs per
batch are summed on the host (plus bo).

Per-core kernel (all matmuls lhsT.T @ rhs, contraction on partitions):
  inputs are host-transposed (xT = x.T, so d_model lands on partitions):
    QT[256,2048] = (Wq_s chunkT).T @ xqT   (accumulate over 8 d-chunks)
    KT, VT likewise.  V = PE-transpose(VT) per head -> V'[Lk, 65]
    (65th column = ones, used to compute the softmax denominator).
  attention per head, scores kept transposed (Lk on partitions):
    ST[128,512] = KT_h_chunk.T @ QT_h_bank          (K = dk = 64)
    P = Exp(ST * (1/sqrt(dk)) + mask_bias)          (one ACT op: scale+mask+exp)
    OT'[65,512] += V'_chunk.T @ P                   (row 64 = denominator)
    OT = OT'[0:64] * broadcast(1/OT'[64])           (DMA partition-broadcast)
  out[2048,1024] = (OT_all chunk).T @ Wo_s  (per-core partial, summed on host)
"""

import sys

for _p in ("/opt/trn_rl_repo",):
    if _p not in sys.path:
        sys.path.insert(0, _p)

import numpy as np

import concourse.bass as bass
import concourse.mybir as mybir
import concourse.tile as tile
from concourse import bacc
from concourse.bass import ts
from concourse.bass_utils import run_bass_kernel_spmd
from concourse.tile_rust import add_dep_helper

F32 = mybir.dt.float32
BF16 = mybir.dt.bfloat16

D_MODEL = 1024
NUM_HEADS = 16
D_K = 64
B = 2
L = 2048
N_CORES = 8
HPC = NUM_HEADS // 4  # heads per core (4)
SCALE = float(np.sqrt(D_K))
MASK_BIAS = -30000.0


def build_nc(L=L, D=D_MODEL, HPC=HPC, mm_dtype=mybir.dt.float32r):
    """Build the per-core Bass program (SPMD: same program, 8 cores).

    Wait-budget discipline (every DMA descriptor and every fp32r matmul
    carries a single hardware sync-wait; compute instructions may carry
    several because walrus splits them):
      * PE writes PSUM, ACT (ScalarE) evacuates PSUM, DVE stays off PSUM,
        so psum slot releases ride the ACT semaphore which the PE already
        tracks through its exp-output waits.
      * DMA-written SBUF slots use bufs=8 so a slot's previous writer sits
        on the same HW ring (WAW implied by ring FIFO); the one remaining
        wait is the readers' engine semaphore.  Explicit order chains keep
        the ring rotation deterministic.
      * softmax normalization broadcasts 1/denom with a K=1 matmul
        (ones.T @ recip_row) instead of a DRAM-bounce DMA.
      * zero "primer" matmuls open each accumulation group so real matmuls
        never join a group start with a data wait.
    """
    DK = D_K
    C = HPC * DK           # attention columns per core (256)
    CT = C // 128          # col tiles (2)
    DC = D // 128          # d_model chunks (8)
    LB = L // 512          # Lq banks (4)
    LT = L // 128          # Lk tiles (16)
    NH = 1                  # one full-width x chunk per d-chunk (bf16: 4KB/row)
    CW = L // NH            # x-chunk width

    MDT = mm_dtype  # dtype for every tensor feeding a matmul

    nc = bacc.Bacc("TRN2", target_bir_lowering=False, debug=False,
                   num_devices=N_CORES)

    xT = {n: nc.dram_tensor(f"x{n}T", [D, L], BF16, kind="ExternalInput").ap()
          for n in ("q", "k", "v")}
    w = {n: nc.dram_tensor(f"w{n}", [D, C], BF16, kind="ExternalInput").ap()
         for n in ("q", "k", "v")}
    wo = nc.dram_tensor("wo", [C, D], BF16, kind="ExternalInput").ap()
    bias = {n: nc.dram_tensor(f"b{n}", [C], F32, kind="ExternalInput").ap()
            for n in ("q", "k", "v")}
    mb = nc.dram_tensor("mb", [128, LT], F32, kind="ExternalInput").ap()
    id2 = nc.dram_tensor("id2", [128, DK], MDT, kind="ExternalInput").ap()
    onesd = nc.dram_tensor("ones", [1, DK], MDT, kind="ExternalInput").ap()
    ztd = nc.dram_tensor("ztc", [128, 128], MDT, kind="ExternalInput").ap()
    vod = nc.dram_tensor("vones", [128, LT * HPC], BF16,
                         kind="ExternalInput").ap()
    out = nc.dram_tensor("partial", [L, D], BF16, kind="ExternalOutput").ap()

    Ident = mybir.ActivationFunctionType.Identity

    with tile.TileContext(nc) as tc:
        with (
            tc.tile_pool(name="consts", bufs=1) as consts,
            tc.tile_pool(name="persist", bufs=1) as persist,
            tc.tile_pool(name="xch", bufs=8) as xch,
            tc.tile_pool(name="work", bufs=6) as work,
            tc.tile_pool(name="pt", bufs=6) as ptp,
            tc.tile_pool(name="ostg", bufs=8) as ostgp,
            tc.tile_pool(name="ps", bufs=8, space="PSUM") as psp,
        ):
            def prime(out_ps, rhs_ap, lhsT=None, start=True, stop=True):
                return nc.tensor.matmul(
                    out_ps, lhsT=(lhsT if lhsT is not None else rhs_ap[:, :1]),
                    rhs=rhs_ap, start=start, stop=stop,
                    skip_group_check=stop is True)

            # PSUM budget (8 banks total, one pool, two tags):
            #   "s2": 2 bufs x [128,1024] f32 (2 banks each) -> 4 banks
            #   "ot": 4 bufs x [128,512]  f32 (1 bank each)  -> 4 banks
            def ps_tile(name=None, dtype=F32):
                return psp.tile([128, 512], dtype, tag="ot", bufs=4,
                                name=name or "pst")

            def ps_wide(name=None, dtype=F32, shape=None):
                return psp.tile(shape or [128, 1024], dtype, tag="s2", bufs=2,
                                name=name or "psw")

            # ---- constants (SWDGE; x/out stay on the HWDGE rings) ----
            w_sb = {}
            for n in ("q", "k", "v"):
                w_sb[n] = consts.tile([128, DC, C], BF16, tag=f"w{n}",
                                      name=f"w{n}_sb")
                nc.gpsimd.dma_start(
                    out=w_sb[n], in_=w[n].rearrange("(c p) n -> p c n", p=128))
            wo_sb = consts.tile([128, CT, D], BF16, tag="wo")
            nc.gpsimd.dma_start(
                out=wo_sb, in_=wo.rearrange("(g p) n -> p g n", p=128))
            b_sb = {}
            for n in ("q", "k", "v"):
                b_sb[n] = consts.tile([128, CT], F32, tag=f"b{n}",
                                      name=f"b{n}_sb")
                nc.gpsimd.dma_start(
                    out=b_sb[n], in_=bias[n].rearrange("(t p) -> p t", p=128))
            mb_sb = consts.tile([128, LT], F32, tag="mb")
            nc.gpsimd.dma_start(out=mb_sb, in_=mb)
            id2_sb = consts.tile([128, DK], MDT, tag="id2")
            nc.gpsimd.dma_start(out=id2_sb, in_=id2)
            v_all = persist.tile([128, LT, HPC, DK + 1], BF16, tag="vall")
            nc.gpsimd.dma_start(
                out=v_all[:, :, :, DK],
                in_=vod.rearrange("p (c h) -> p c h", c=LT))
            ones_sb = consts.tile([1, DK], MDT, tag="ones")
            nc.gpsimd.dma_start(out=ones_sb, in_=onesd)

            # ---- phase 1: projections -> QT/KT/VT [128, CT, L] ----
            # accumulators: ct=0 -> two [128,1024] "s2" tiles (lb pairs),
            # ct=1 -> four [128,512] "ot" tiles; 8 banks total.
            projT = {}
            last_xdma = None
            for n in ("q", "k", "v"):
                # q/k land in bf16 (the score matmul runs 2-3x faster than
                # f32r on hw); v stays f32r for the PE transpose, rounded to
                # bf16 at the v_all evacuation.
                dst = persist.tile([128, CT, L], MDT if n == "v" else BF16,
                                   tag=f"{n}t", name=f"{n}t_sb")
                projT[n] = dst
                wide = [ps_wide(name=f"pw_{n}_{i}") for i in range(2)]
                narrow = [ps_tile(name=f"pn_{n}_{i}") for i in range(LB)]

                def acc_ap(ct, lb):
                    if ct == 0:
                        return wide[lb // 2][:, ts(lb % 2, 512)]
                    return narrow[lb]

                for hc in range(DC * NH):
                    dc, lh = hc // NH, hc % NH
                    xc = xch.tile([128, CW], BF16, tag="xc")
                    xd = nc.sync.dma_start(
                        out=xc, in_=xT[n][ts(dc, 128), ts(lh, CW)])
                    if last_xdma is not None:
                        # deterministic ring rotation (slot i <-> ring i)
                        add_dep_helper(xd.ins, last_xdma.ins, sync=False,
                                       reason="xdma-order")
                    last_xdma = xd
                    for ct in range(CT):
                        for lb2 in range(CW // 512):
                            lb = lh * (CW // 512) + lb2
                            nc.tensor.matmul(
                                acc_ap(ct, lb),
                                lhsT=w_sb[n][:, dc, ts(ct, 128)],
                                rhs=xc[:, ts(lb2, 512)],
                                start=(dc == 0), stop=(dc == DC - 1))
                for lbp in range(2):
                    nc.scalar.activation(
                        dst[:, 0, ts(lbp, 1024)], wide[lbp], Ident,
                        bias=b_sb[n][:, 0:1])
                for lb in range(LB):
                    nc.scalar.activation(
                        dst[:, 1, ts(lb, 512)], narrow[lb], Ident,
                        bias=b_sb[n][:, 1:2])

            # ---- phase 1b: V' = [V | ones] in natural [Lk, 65] layout ----
            for h in range(HPC):
                g, po = h // 2, 64 * (h % 2)
                for c in range(LT):
                    pst = ps_tile(name="pst", dtype=MDT)
                    nc.tensor.transpose(
                        pst[:, :DK],
                        projT["v"][po:po + DK, g, ts(c, 128)],
                        id2_sb[po:po + DK, :])
                    nc.vector.tensor_copy(v_all[:, c, h, 0:DK], pst[:, :DK])

            # ---- phase 2: attention, head-pair packed ----
            # The two heads of a g-group sit on partitions 0-63 / 64-127 of
            # projT, so their K=64 score matmuls carry tile_position (0,0) /
            # (64,0) and execute CONCURRENTLY in the PE array (row tiling).
            # A half-array matmul stream reads as low PE activity and leaves
            # the HAM clock gate throttled at 1.2 GHz; the packed pair fills
            # all 128 rows.  One 1024-wide EXP covers both heads (the mask
            # bias depends only on the k-chunk c, not the head).
            ot_sb = persist.tile([128, CT, L], BF16, tag="ot")
            Exp = mybir.ActivationFunctionType.Exp

            def tail_pre(otp):
                # Normalization front half (all DVE): pull the denominator
                # row and O to SBUF (releasing the psum bank), ~51-ULP
                # reciprocal, round to f32r for the broadcast matmul.
                dn = work.tile([1, 512], F32, tag="dn")
                nc.vector.tensor_copy(dn, otp[DK:DK + 1, :])
                ot_c = work.tile([64, 512], F32, tag="otc")
                nc.vector.tensor_copy(ot_c, otp[0:DK, :])
                rc = work.tile([1, 512], F32, tag="rc")
                nc.vector.reciprocal_approx_fast(rc, dn)
                rc_r = work.tile([1, 512], MDT, tag="rcr")
                with nc.allow_low_precision(
                        reason="~18-bit reciprocal feeds the "
                               "broadcast matmul; |denom|>=1"):
                    nc.vector.tensor_copy(rc_r, rc)
                return ot_c, rc_r

            def tail_post(g, lb, po, otp, ot_c, rc_r):
                # Back half: broadcast 1/denom into the already-copied-out
                # psum bank with a K=1 matmul, multiply on DVE into ot_sb.
                # Deferred past the next pass's first EXP so the in-order PE
                # queue never stalls waiting on the DVE reciprocal chain.
                nc.tensor.matmul(otp[:DK, :], lhsT=ones_sb,
                                 rhs=rc_r, start=True, stop=True)
                nc.vector.tensor_mul(
                    ot_sb[po:po + DK, g, ts(lb, 512)],
                    ot_c, otp[0:DK, :])

            # phase 3 is emitted in (t, half) units; the g=1 passes interleave
            # the units for the previous lb chunk into their PE/DVE slack
            # (the pass schedule is ACT-bound), leaving only the last chunk
            # exposed after the attention loop.
            WOW = min(512, D)
            p3_state = {"last_odma": None, "ostg": {}}

            def p3_unit(t, half):
                wps = ps_tile(name="wps")
                for gg in range(CT):
                    nc.tensor.matmul(
                        wps[:, :WOW], lhsT=ot_sb[:, gg, ts(t, 128)],
                        rhs=wo_sb[:, gg, ts(half, WOW)],
                        start=(gg == 0), stop=(gg == CT - 1))
                if t not in p3_state["ostg"]:
                    p3_state["ostg"][t] = ostgp.tile(
                        [128, D], BF16, tag="os", name="ostg")
                # alternate the psum evacuation between ACT and DVE so the
                # two engines drain the 2-slot wps ring in parallel
                dst = p3_state["ostg"][t][:, ts(half, WOW)]
                if half % 2 == 0:
                    nc.vector.tensor_copy(dst, wps[:, :WOW])
                else:
                    nc.scalar.copy(out=dst, in_=wps[:, :WOW])
                if half == D // WOW - 1:
                    od = nc.sync.dma_start(out=out[ts(t, 128), :],
                                           in_=p3_state["ostg"].pop(t))
                    if p3_state["last_odma"] is not None:
                        add_dep_helper(od.ins, p3_state["last_odma"].ins,
                                       sync=False, reason="odma-order")
                    p3_state["last_odma"] = od

            pending = []
            for g in range(CT):
                hA, hB = 2 * g, 2 * g + 1
                for lb in range(LB):
                    otA = ps_tile(name="otA")
                    otB = ps_tile(name="otB")
                    pend = None
                    for c in range(LT):
                        s2 = ps_wide(name="s2")
                        nc.tensor.matmul(
                            s2[:, 0:512],
                            lhsT=projT["k"][0:DK, g, ts(c, 128)],
                            rhs=projT["q"][0:DK, g, ts(lb, 512)],
                            start=True, stop=True)
                        nc.tensor.matmul(
                            s2[:, 512:1024],
                            lhsT=projT["k"][DK:2 * DK, g, ts(c, 128)],
                            rhs=projT["q"][DK:2 * DK, g, ts(lb, 512)],
                            start=True, stop=True)
                        p2 = ptp.tile([128, 1024], BF16, tag="p")
                        nc.scalar.activation(
                            p2, s2, Exp,
                            bias=mb_sb[:, c:c + 1], scale=1.0 / SCALE)
                        if c == 1:
                            for args in pending:
                                tail_post(*args)
                            pending = []
                        if pend is not None:
                            c0, p0 = pend
                            nc.tensor.matmul(
                                otA[:DK + 1, :],
                                lhsT=v_all[:, c0, hA, :], rhs=p0[:, 0:512],
                                start=(c0 == 0), stop=(c0 == LT - 1))
                            nc.tensor.matmul(
                                otB[:DK + 1, :],
                                lhsT=v_all[:, c0, hB, :], rhs=p0[:, 512:1024],
                                start=(c0 == 0), stop=(c0 == LT - 1))
                        pend = (c, p2)
                    c0, p0 = pend
                    nc.tensor.matmul(
                        otA[:DK + 1, :], lhsT=v_all[:, c0, hA, :],
                        rhs=p0[:, 0:512], start=(c0 == 0), stop=True)
                    nc.tensor.matmul(
                        otB[:DK + 1, :], lhsT=v_all[:, c0, hB, :],
                        rhs=p0[:, 512:1024], start=(c0 == 0), stop=True)
                    for po, otp in ((0, otA), (64, otB)):
                        ot_c, rc_r = tail_pre(otp)
                        pending.append((g, lb, po, otp, ot_c, rc_r))
            for args in pending:
                tail_post(*args)
            for t3 in range(LT):
                for h3 in range(D // WOW):
                    p3_unit(t3, h3)


    nc.compile()   # bacc lowering: event sems split multi-wait instructions
    return nc


def _strip_implied_dma_ring_waits(nc):
    """Drop DMA ring-semaphore waits that are implied by a compute-engine
    wait on the same descriptor.

    A recycled DMA-written SBUF slot gets two waits: the readers' engine
    semaphore (slot release) and the previous writer's DMA-ring semaphore
    (WAW).  The readers themselves data-waited on that previous DMA, so
    release >= WAW always; but DMA descriptors carry a single hardware
    sync-wait, so Tile's conservative pair fails walrus codegen.  Keep the
    engine wait, drop the ring wait.  Applied only to the x-chunk loads and
    output-staging stores, whose only DMA-semaphore deps are these WAW /
    WAR-release edges (their data comes from DRAM inputs or compute-engine
    writes, never from another DMA).
    """
    import concourse.mybir as _mb
    for ins in nc.inst_map.values():
        if type(ins).__name__ != "InstDMACopy":
            continue
        if not ins.outs:
            continue
        memref = getattr(ins.outs[0], "memref", "") or ""
        src_ref = getattr(ins.ins[0], "memref", "") if ins.ins else ""
        if not (memref.startswith("xc_") or (src_ref or "").startswith("ostg")):
            continue
        si = ins.sync_info
        if not si or not si.on_wait or len(si.on_wait) < 2:
            continue
        eng = [w for w in si.on_wait
               if not (w.ant_name or "").startswith(("DMAHW", "DMASW"))]
        if not eng:
            continue
        ins.sync_info = _mb.SyncInfo(on_wait=eng, on_update=list(si.on_update))


def make_in_maps(query, key, value, mask, Wq, bq, Wk, bk, Wv, bv, Wo, bo,
                 L=L, D=D_MODEL, HPC=HPC):
    """Host-side sharding: per-core input dicts."""
    DK = D_K
    C = HPC * DK
    LT = L // 128
    import ml_dtypes
    bf16 = ml_dtypes.bfloat16
    id2 = np.ascontiguousarray(
        np.tile(np.eye(DK, dtype=np.float32), (2, 1)))
    ones = np.ones((1, DK), np.float32)
    ztc = np.zeros((128, 128), np.float32)
    vones = np.ones((128, (L // 128) * HPC), bf16)
    in_maps = []
    xTs = {}
    mbs = {}
    n_cores = (query.shape[0]) * (D // C)
    groups_per_batch = D // C
    for b in range(query.shape[0]):
        xTs[b] = {
            "q": np.ascontiguousarray(query[b].T).astype(bf16),
            "k": np.ascontiguousarray(key[b].T).astype(bf16),
            "v": np.ascontiguousarray(value[b].T).astype(bf16),
        }
        mbf = np.where(mask[b, 0], np.float32(MASK_BIAS), np.float32(0.0))
        mbs[b] = np.ascontiguousarray(
            mbf.reshape(LT, 128).T.astype(np.float32))
    for c in range(n_cores):
        b, hg = divmod(c, groups_per_batch)
        sl = slice(hg * C, (hg + 1) * C)
        in_maps.append({
            "xqT": xTs[b]["q"], "xkT": xTs[b]["k"], "xvT": xTs[b]["v"],
            "wq": np.ascontiguousarray(Wq[:, sl]).astype(bf16),
            "wk": np.ascontiguousarray(Wk[:, sl]).astype(bf16),
            "wv": np.ascontiguousarray(Wv[:, sl]).astype(bf16),
            "wo": np.ascontiguousarray(Wo[sl, :]).astype(bf16),
            "bq": np.ascontiguousarray(bq[sl]),
            "bk": np.ascontiguousarray(bk[sl]),
            "bv": np.ascontiguousarray(bv[sl]),
            "mb": mbs[b],
            "id2": id2,
            "ones": ones, "ztc": ztc, "vones": vones,
        })
    return in_maps


_NC_CACHE = {}


def _get_nc(mm_dtype=mybir.dt.float32r):
    key = str(mm_dtype)
    if key not in _NC_CACHE:
        _NC_CACHE[key] = build_nc(mm_dtype=mm_dtype)
    return _NC_CACHE[key]


def run(inputs, mm_dtype=mybir.dt.float32r, trace=False):
    """Run on 8 cores; returns (full_output, BassKernelResults)."""
    inputs = {k: np.asarray(v) for k, v in inputs.items()}
    nc = _get_nc(mm_dtype)
    in_maps = make_in_maps(**inputs)
    res = run_bass_kernel_spmd(nc, in_maps, list(range(N_CORES)), trace=trace)
    groups_per_batch = N_CORES // B
    out = np.zeros((B, L, D_MODEL), np.float32)
    for b in range(B):
        acc = np.zeros((L, D_MODEL), np.float32)
        for hg in range(groups_per_batch):
            acc += res.results[b * groups_per_batch + hg]["partial"].astype(
                np.float32)
        out[b] = acc + inputs["bo"][None, :]
    return out, res


def kernel(**inputs) -> np.ndarray:
    out, _ = run(inputs)
    return out

